# revision 6
# baseline (speedup 1.0000x reference)
"""Bundle-adjustment projection kernel for Trainium2 (8 NeuronCores).

Strategy (per spec sharding freedom): edges are globally sorted by map-point id
(host-side shard permutation) and sharded contiguously across 8 cores x 8
GPSIMD Q7 groups.  Point data is "replicated" host-side once per point-run
(the blessed replicate-the-small-tMP operation) into a sparse run-start value
stream V; the device expands runs with a DVE prefix scan.  Keyframe poses are
fetched per-slot on-device with GPSIMD ap_gather from a feature-transposed
replicated tKF table.  A PE matmul against a fixed selector matrix folds the
4x4 matvec row-reduction AND the intrinsics (FX,FY,CX,CY) into numerator /
denominator rows; DVE reciprocal+multiply gives pixel coords; host inverts the
shard permutation.
"""
import sys
sys.path.insert(0, "/opt/trn_rl_repo")

import numpy as np

FX, FY, CX, CY = 320.0, 320.0, 320.0, 240.0
N_MP, N_KF, M = 200000, 2000, 4000000
N_CORES = 8
N_GROUPS = 8                      # Q7 groups per core
CHUNK = 2048                      # scan / gather / psum-pack unit (cols)
SUBCH = 512                       # matmul free-dim tile
N_CHUNKS = 31
SPG = CHUNK * N_CHUNKS            # 63488 slots per group
IDXW = SPG // 16                  # 3968
SLOTS_CORE = N_GROUPS * SPG       # 507904
SLOTS_TOTAL = N_CORES * SLOTS_CORE  # 4063232
OUT_ROWS = N_CHUNKS * 4 * 16          # 1984

_CACHE = {}


def _build(n_rep=1):
    import concourse.bacc as bacc
    import concourse.mybir as mybir
    import concourse.tile as tile

    f32 = mybir.dt.float32
    i16 = mybir.dt.int16
    Alu = mybir.AluOpType

    nc = bacc.Bacc(None, target_bir_lowering=False)
    tbl_h = nc.dram_tensor("tbl", [128, N_KF], f32, kind="ExternalInput")
    s_h = nc.dram_tensor("S", [128, 64], f32, kind="ExternalInput")
    kf_h = nc.dram_tensor("kf16", [128, IDXW], i16, kind="ExternalInput")
    v_h = nc.dram_tensor("V", [128, SPG], f32, kind="ExternalInput")
    out_h = nc.dram_tensor("out", [OUT_ROWS, SUBCH], f32, kind="ExternalOutput")

    with tile.TileContext(nc) as tc:
        with (
            tc.tile_pool(name="const", bufs=1) as constp,
            tc.tile_pool(name="work", bufs=3) as work,
            tc.tile_pool(name="psum", bufs=8, space="PSUM") as psump,
        ):
            tblt = constp.tile([128, N_KF], f32)
            nc.sync.dma_start(tblt[:], tbl_h[:])
            st = constp.tile([128, 64], f32)
            nc.sync.dma_start(st[:], s_h[:])
            idxt = constp.tile([128, IDXW], i16)
            nc.sync.dma_start(idxt[:], kf_h[:])
            for _rep in range(n_rep):
                for t in range(N_CHUNKS):
                    c0 = t * CHUNK
                    vt = work.tile([128, CHUNK], f32, tag="v")
                    nc.sync.dma_start(vt[:], v_h[:, c0:c0 + CHUNK])
                    pg = work.tile([128, CHUNK], f32, tag="pg")
                    nc.gpsimd.ap_gather(
                        pg[:], tblt[:], idxt[:, t * 128:(t + 1) * 128],
                        channels=128, num_elems=N_KF, d=1, num_idxs=CHUNK)
                    mk = work.tile([128, CHUNK], f32, tag="mk")
                    nc.vector.tensor_scalar(mk[:], vt[:], 0.0, None, op0=Alu.is_equal)
                    mg = work.tile([128, CHUNK], f32, tag="mg")
                    nc.vector.tensor_tensor_scan(
                        mg[:], mk[:], vt[:], 0.0, op0=Alu.mult, op1=Alu.add)
                    mprod = work.tile([128, CHUNK], f32, tag="mprod")
                    nc.vector.tensor_tensor(mprod[:], pg[:], mg[:], op=Alu.mult)
                    for b in range(4):
                        rhs = mprod[:, b * SUBCH:(b + 1) * SUBCH]
                        ps = psump.tile([64, SUBCH], f32, tag="ps")
                        nc.tensor.matmul(
                            out=ps[:, :], lhsT=st[:, 0:64],
                            rhs=rhs, start=True, stop=True)
                        rec = work.tile([16, SUBCH], f32, tag="rec")
                        nc.vector.reciprocal(rec[:], ps[32:48, :])
                        xy16 = work.tile([16, SUBCH], f32, tag="xy16")
                        nc.vector.tensor_tensor(
                            xy16[:, :], ps[0:16, :], rec[:], op=Alu.mult)
                        r0 = 16 * (t * 4 + b)
                        nc.sync.dma_start(out_h[r0:r0 + 16, :], xy16[:, :])
    nc.finalize()
    return nc


def _selector():
    S = np.zeros((128, 64), np.float32)
    for q in range(N_GROUPS):
        for i, (F, C) in enumerate([(FX, CX), (FY, CY)]):
            j = 2 * q + i
            S[16 * q + 4 * i:16 * q + 4 * i + 4, j] = F
            S[16 * q + 8:16 * q + 12, j] += C
            S[16 * q + 8:16 * q + 12, 32 + 2 * q + i] = 1.0
    return S


def _prep_inputs(tMP, tKF, kf_ids, mp_ids, idxKF, idxMP):
    idsKF = np.searchsorted(np.asarray(idxKF), np.asarray(kf_ids))
    idsMP = np.searchsorted(np.asarray(idxMP), np.asarray(mp_ids))
    perm = np.argsort(idsMP, kind="stable")
    mp_s = idsMP[perm]
    kf_s = idsKF[perm]

    kf_pad = np.zeros(SLOTS_TOTAL, np.int16)
    kf_pad[:M] = kf_s.astype(np.int16)

    starts = np.ones(SLOTS_TOTAL, bool)
    starts[1:M] = mp_s[1:] != mp_s[:-1]
    jcol = np.arange(SLOTS_TOTAL) % SPG
    starts |= (jcol % CHUNK) == 0

    tMPh = np.concatenate(
        [np.asarray(tMP, np.float32), np.ones((N_MP, 1), np.float32)], axis=1)
    Vflat = np.zeros((SLOTS_TOTAL, 4), np.float32)
    sidx = np.nonzero(starts)[0]
    vals = np.ones((len(sidx), 4), np.float32)
    in_edge = sidx < M
    vals[in_edge] = tMPh[mp_s[sidx[in_edge]]]
    Vflat[sidx] = vals

    tblv = np.ascontiguousarray(
        np.tile(np.asarray(tKF, np.float32).reshape(N_KF, 16).T, (N_GROUPS, 1)))
    S = _selector()

    in_maps = []
    for c in range(N_CORES):
        seg = slice(c * SLOTS_CORE, (c + 1) * SLOTS_CORE)
        kfc = kf_pad[seg].reshape(N_GROUPS, SPG)
        kf_w = np.ascontiguousarray(
            kfc.reshape(N_GROUPS, IDXW, 16).transpose(0, 2, 1).reshape(128, IDXW))
        Vc = Vflat[seg].reshape(N_GROUPS, SPG, 4)
        Vc16 = np.ascontiguousarray(
            np.tile(Vc.transpose(0, 2, 1), (1, 4, 1)).reshape(128, SPG))
        in_maps.append({"tbl": tblv, "S": S, "kf16": kf_w, "V": Vc16})
    return in_maps, perm


def _unshard(outs, perm):
    # outs: [N_CORES][128, OUT_W]
    r = np.arange(M)
    c = r // SLOTS_CORE
    rr = r % SLOTS_CORE
    q = rr // SPG
    jj = rr % SPG
    sub = jj // SUBCH
    jc = jj % SUBCH
    px = 16 * sub + 2 * q
    stacked = np.stack(outs)  # [8, OUT_ROWS, SUBCH]
    res = np.empty((M, 2), np.float32)
    res[perm, 0] = stacked[c, px, jc]
    res[perm, 1] = stacked[c, px + 1, jc]
    return res


def kernel(tMP, tKF, kf_ids, mp_ids, idxKF, idxMP):
    from concourse.bass_utils import run_bass_kernel_spmd

    if "nc" not in _CACHE:
        _CACHE["nc"] = _build()
    nc = _CACHE["nc"]
    in_maps, perm = _prep_inputs(tMP, tKF, kf_ids, mp_ids, idxKF, idxMP)
    res = run_bass_kernel_spmd(nc, in_maps, core_ids=list(range(N_CORES)))
    outs = [res.results[i]["out"] for i in range(N_CORES)]
    return _unshard(outs, perm)
